# revision 4
# baseline (speedup 1.0000x reference)
"""RBF kernel feature map: out[b, r] = exp(-||x[b] - refs[r]||^2).

Computed via the GEMM expansion on 8 NeuronCores, data-parallel over the
batch dim of x (2048 rows per core), refs replicated.

Per-core device kernel, one K=68 matmul per [128, 512] PSUM bank:
    psum[b, r] = 2*sum_d x[b,d]*refs[r,d] - r_sq[r] - x_sq[b]
               = -||x[b] - refs[r]||^2          (the 2x is folded into
    out[b, r]  = exp(psum[b, r])                 the packed x rows)

All operands are fp16 (halves input DMA + SBUF traffic, full-rate PE);
the norm terms are split hi (fp16-representable) + lo (remainder) so the
~30..120 magnitude norm terms stay accurate.  The Exp activation covers
a full [128, 2048] 4-bank PSUM span per instruction (amortizes the
~0.3us per-instruction ACT overhead) and writes bf16, halving the
dominant output HBM traffic (the host upcasts to f32).

The input load is split across the three HWDGE queues (sync / vector /
scalar sequencers): SWDGE (gpsimd) descriptor generation for this
pattern measured ~7us before the last packet landed, stalling the first
matmul until ~15us.  Measured rel err vs the fp64 reference ~4e-3
against a 2e-2 gate.

Uses bacc.Bacc (not raw bass.Bass): TRN2 instructions carry at most one
semaphore wait, and Bacc.compile()'s generate_event_semaphores pass
legalizes the multi-wait instructions Tile emits.
"""

import numpy as np

N_CORES = 8
B, D, R = 16384, 64, 2048
B_SHARD = B // N_CORES  # 2048
K = D + 4  # 64 data rows + r_sq hi/lo + x_sq hi/lo rows
BT = 128  # batch rows per tile (PSUM partition dim)
RC = 512  # refs cols per matmul (one fp32 PSUM bank)
ACT_COLS = 2048  # Exp activation span: 4 PSUM banks per instruction


def _build_nc():
    from contextlib import ExitStack

    import concourse.tile as tile
    from concourse import bacc, mybir

    f16 = mybir.dt.float16
    bf16 = mybir.dt.bfloat16
    f32 = mybir.dt.float32

    nc = bacc.Bacc(None)
    # x-shard transpose and refs transpose concatenated along the free dim:
    # cols [0, B_SHARD) = (2x).T block, cols [B_SHARD, B_SHARD+R) = refs.T.
    inT_aug = nc.declare_dram_parameter(
        "inT_aug", [K, B_SHARD + R], f16, isOutput=False
    )
    out = nc.declare_dram_parameter("out", [B_SHARD, R], bf16, isOutput=True)

    n_bt = B_SHARD // BT
    n_rc = R // RC

    with tile.TileContext(nc) as tc, ExitStack() as ctx:
        consts = ctx.enter_context(tc.tile_pool(name="consts", bufs=1))
        outs = ctx.enter_context(tc.tile_pool(name="outs", bufs=4))
        psums = ctx.enter_context(tc.tile_pool(name="psums", bufs=2, space="PSUM"))

        in_sb = consts.tile([K, B_SHARD + R], f16)
        # Input load split across the two HWDGE queues (sync=SP, scalar=ACT;
        # DVE has no DGE) so transfers run in parallel and the bt=0 matmuls
        # (x block 0 + refs halves) release as early as possible; the x
        # remainder only gates bt>=1.
        nc.scalar.dma_start(out=in_sb[:, :BT], in_=inT_aug[:, :BT])
        nc.sync.dma_start(
            out=in_sb[:, B_SHARD : B_SHARD + R // 2],
            in_=inT_aug[:, B_SHARD : B_SHARD + R // 2],
        )
        nc.scalar.dma_start(
            out=in_sb[:, B_SHARD + R // 2 :], in_=inT_aug[:, B_SHARD + R // 2 :]
        )
        nc.sync.dma_start(out=in_sb[:, BT:B_SHARD], in_=inT_aug[:, BT:B_SHARD])

        for bt in range(n_bt):
            ps = psums.tile([BT, ACT_COLS], f32)
            out_sb = outs.tile([BT, R], bf16)
            for rc in range(n_rc):
                nc.tensor.matmul(
                    ps[:, rc * RC : (rc + 1) * RC],
                    lhsT=in_sb[:, bt * BT : (bt + 1) * BT],
                    rhs=in_sb[:, B_SHARD + rc * RC : B_SHARD + (rc + 1) * RC],
                    start=True,
                    stop=True,
                )
            nc.scalar.activation(
                out_sb,
                ps,
                mybir.ActivationFunctionType.Exp,
                bias=0.0,
                scale=1.0,
            )
            nc.sync.dma_start(out=out[bt * BT : (bt + 1) * BT, :], in_=out_sb)

    nc.compile()
    return nc


def _hi_lo(v):
    """Split fp64 vector into fp16-representable hi + fp16 remainder lo."""
    hi = v.astype(np.float16)
    lo = (v - hi.astype(np.float64)).astype(np.float16)
    return hi, lo


def make_in_maps(x, refs):
    """Host-side prep: shard/transpose x, pack norm terms as extra K rows.

    The x data rows carry 2x so the single K=68 contraction yields
    2*x.r - x_sq - r_sq = -||x - r||^2 directly.
    """
    x = np.ascontiguousarray(x, dtype=np.float32)
    refs = np.ascontiguousarray(refs, dtype=np.float32)

    r_hi, r_lo = _hi_lo((refs.astype(np.float64) ** 2).sum(axis=1))
    x_sq = (x.astype(np.float64) ** 2).sum(axis=1)  # [B]
    xT16 = np.ascontiguousarray((2.0 * x.T).astype(np.float16))  # [D, B]
    rT16 = np.ascontiguousarray(refs.T.astype(np.float16))  # [D, R]

    in_maps = []
    for c in range(N_CORES):
        sl = slice(c * B_SHARD, (c + 1) * B_SHARD)
        x_hi, x_lo = _hi_lo(x_sq[sl])
        inT_aug = np.empty((K, B_SHARD + R), np.float16)
        inT_aug[:D, :B_SHARD] = xT16[:, sl]
        inT_aug[D, :B_SHARD] = 1.0
        inT_aug[D + 1, :B_SHARD] = 1.0
        inT_aug[D + 2, :B_SHARD] = -x_hi
        inT_aug[D + 3, :B_SHARD] = -x_lo
        inT_aug[:D, B_SHARD:] = rT16
        inT_aug[D, B_SHARD:] = -r_hi
        inT_aug[D + 1, B_SHARD:] = -r_lo
        inT_aug[D + 2, B_SHARD:] = 1.0
        inT_aug[D + 3, B_SHARD:] = 1.0
        in_maps.append({"inT_aug": inT_aug})
    return in_maps


_NC_CACHE = None


def get_nc():
    global _NC_CACHE
    if _NC_CACHE is None:
        _NC_CACHE = _build_nc()
    return _NC_CACHE


def kernel(x, refs):
    from concourse.bass_utils import run_bass_kernel_spmd

    in_maps = make_in_maps(x, refs)
    res = run_bass_kernel_spmd(
        get_nc(), in_maps, core_ids=list(range(N_CORES))
    ).results
    return np.concatenate(
        [res[c]["out"].astype(np.float32) for c in range(N_CORES)], axis=0
    )


# revision 5
# speedup vs baseline: 1.1905x; 1.1905x over previous
"""RBF kernel feature map: out[b, r] = exp(-||x[b] - refs[r]||^2).

Computed via the GEMM expansion on 8 NeuronCores, data-parallel over the
batch dim of x (2048 rows per core), refs replicated.

Per-core device kernel, one K=68 matmul per [128, 512] PSUM bank:
    psum[b, r] = 2*sum_d x[b,d]*refs[r,d] - r_sq[r] - x_sq[b]
               = -||x[b] - refs[r]||^2          (the 2x is folded into
    out[b, r]  = exp(psum[b, r])                 the packed x rows)

All operands are fp16 (halves input DMA + SBUF traffic, full-rate PE);
the norm terms are split hi (fp16-representable) + lo (remainder) so the
~30..120 magnitude norm terms stay accurate.  The Exp activation covers
a full [128, 2048] 4-bank PSUM span per instruction (amortizes the
~0.6us per-instruction ACT overhead) and writes bf16, halving the
dominant output HBM traffic (the host upcasts to f32).

Input DMA: the K=68 rows are padded to 128 DRAM rows because the DMA
engine spread is driven by the partition count — a [68, n] transfer
lands on ~4 of the 16 SDMA engines (~100GB/s), while [128, n] uses all
16 (~400GB/s).  The matmuls still address only the first 68 partitions.
Both input pieces go on the sync (SP) HWDGE queue: SWDGE (gpsimd)
descriptor generation measured ~6us for this pattern, and HWDGE rings
opened on the scalar queue tax every ACT instruction by ~0.4us.

Measured rel err vs the fp64 reference ~4e-3 against a 2e-2 gate.

Uses bacc.Bacc (not raw bass.Bass): TRN2 instructions carry at most one
semaphore wait, and Bacc.compile()'s generate_event_semaphores pass
legalizes the multi-wait instructions Tile emits.
"""

import numpy as np

N_CORES = 8
B, D, R = 16384, 64, 2048
B_SHARD = B // N_CORES  # 2048
K = D + 4  # 64 data rows + r_sq hi/lo + x_sq hi/lo rows
KP = 128  # K padded to full partition count for 16-engine DMA spread
BT = 128  # batch rows per tile (PSUM partition dim)
RC = 512  # refs cols per matmul (one fp32 PSUM bank)
ACT_COLS = 2048  # Exp activation span: 4 PSUM banks per instruction


def _build_nc():
    from contextlib import ExitStack

    import concourse.tile as tile
    from concourse import bacc, mybir

    f16 = mybir.dt.float16
    bf16 = mybir.dt.bfloat16
    f32 = mybir.dt.float32

    nc = bacc.Bacc(None)
    # x-shard transpose and refs transpose concatenated along the free dim:
    # cols [0, B_SHARD) = (2x).T block, cols [B_SHARD, B_SHARD+R) = refs.T.
    inT_aug = nc.declare_dram_parameter(
        "inT_aug", [KP, B_SHARD + R], f16, isOutput=False
    )
    out = nc.declare_dram_parameter("out", [B_SHARD, R], bf16, isOutput=True)

    n_bt = B_SHARD // BT
    n_rc = R // RC

    with tile.TileContext(nc) as tc, ExitStack() as ctx:
        consts = ctx.enter_context(tc.tile_pool(name="consts", bufs=1))
        outs = ctx.enter_context(tc.tile_pool(name="outs", bufs=4))
        psums = ctx.enter_context(tc.tile_pool(name="psums", bufs=2, space="PSUM"))

        in_sb = consts.tile([KP, B_SHARD + R], f16)
        # refs piece first (gates every matmul), x piece second (gates bt>=1
        # only beyond the first block, which rides with the refs transfer).
        nc.sync.dma_start(out=in_sb[:, B_SHARD:], in_=inT_aug[:, B_SHARD:])
        nc.sync.dma_start(out=in_sb[:, :B_SHARD], in_=inT_aug[:, :B_SHARD])

        for bt in range(n_bt):
            ps = psums.tile([BT, ACT_COLS], f32)
            out_sb = outs.tile([BT, R], bf16)
            for rc in range(n_rc):
                nc.tensor.matmul(
                    ps[:, rc * RC : (rc + 1) * RC],
                    lhsT=in_sb[:K, bt * BT : (bt + 1) * BT],
                    rhs=in_sb[:K, B_SHARD + rc * RC : B_SHARD + (rc + 1) * RC],
                    start=True,
                    stop=True,
                )
            nc.scalar.activation(
                out_sb,
                ps,
                mybir.ActivationFunctionType.Exp,
                bias=0.0,
                scale=1.0,
            )
            nc.sync.dma_start(out=out[bt * BT : (bt + 1) * BT, :], in_=out_sb)

    nc.compile()
    return nc


def _hi_lo(v):
    """Split fp64 vector into fp16-representable hi + fp16 remainder lo."""
    hi = v.astype(np.float16)
    lo = (v - hi.astype(np.float64)).astype(np.float16)
    return hi, lo


def make_in_maps(x, refs):
    """Host-side prep: shard/transpose x, pack norm terms as extra K rows.

    The x data rows carry 2x so the single K=68 contraction yields
    2*x.r - x_sq - r_sq = -||x - r||^2 directly.
    """
    x = np.ascontiguousarray(x, dtype=np.float32)
    refs = np.ascontiguousarray(refs, dtype=np.float32)

    r_hi, r_lo = _hi_lo((refs.astype(np.float64) ** 2).sum(axis=1))
    x_sq = (x.astype(np.float64) ** 2).sum(axis=1)  # [B]
    xT16 = np.ascontiguousarray((2.0 * x.T).astype(np.float16))  # [D, B]
    rT16 = np.ascontiguousarray(refs.T.astype(np.float16))  # [D, R]

    in_maps = []
    for c in range(N_CORES):
        sl = slice(c * B_SHARD, (c + 1) * B_SHARD)
        x_hi, x_lo = _hi_lo(x_sq[sl])
        inT_aug = np.zeros((KP, B_SHARD + R), np.float16)
        inT_aug[:D, :B_SHARD] = xT16[:, sl]
        inT_aug[D, :B_SHARD] = 1.0
        inT_aug[D + 1, :B_SHARD] = 1.0
        inT_aug[D + 2, :B_SHARD] = -x_hi
        inT_aug[D + 3, :B_SHARD] = -x_lo
        inT_aug[:D, B_SHARD:] = rT16
        inT_aug[D, B_SHARD:] = -r_hi
        inT_aug[D + 1, B_SHARD:] = -r_lo
        inT_aug[D + 2, B_SHARD:] = 1.0
        inT_aug[D + 3, B_SHARD:] = 1.0
        in_maps.append({"inT_aug": inT_aug})
    return in_maps


_NC_CACHE = None


def get_nc():
    global _NC_CACHE
    if _NC_CACHE is None:
        _NC_CACHE = _build_nc()
    return _NC_CACHE


def kernel(x, refs):
    from concourse.bass_utils import run_bass_kernel_spmd

    in_maps = make_in_maps(x, refs)
    res = run_bass_kernel_spmd(
        get_nc(), in_maps, core_ids=list(range(N_CORES))
    ).results
    return np.concatenate(
        [res[c]["out"].astype(np.float32) for c in range(N_CORES)], axis=0
    )


# revision 6
# speedup vs baseline: 1.2442x; 1.0452x over previous
"""RBF kernel feature map: out[b, r] = exp(-||x[b] - refs[r]||^2).

Computed via the GEMM expansion on 8 NeuronCores, data-parallel over the
batch dim of x (2048 rows per core), refs replicated.

Per-core device kernel, one K=68 matmul per [128, 512] PSUM bank:
    psum[b, r] = 2*sum_d x[b,d]*refs[r,d] - r_sq[r] - x_sq[b]
               = -||x[b] - refs[r]||^2          (the 2x is folded into
    out[b, r]  = exp(psum[b, r])                 the packed x rows)

All operands are fp16 (halves input DMA + SBUF traffic, full-rate PE);
the norm terms are split hi (fp16-representable) + lo (remainder) so the
~30..120 magnitude norm terms stay accurate.  The Exp activation covers
a full [128, 2048] 4-bank PSUM span per instruction (amortizes the
~0.6us per-instruction ACT overhead) and writes bf16, halving the
dominant output HBM traffic (the host upcasts to f32).

Input DMA: the K=68 rows are padded to 128 DRAM rows because the DMA
engine spread is driven by the partition count — a [68, n] transfer
lands on ~4 of the 16 SDMA engines (~100GB/s), while [128, n] uses all
16 (~400GB/s).  The matmuls still address only the first 68 partitions.
Both input pieces go on the sync (SP) HWDGE queue: SWDGE (gpsimd)
descriptor generation measured ~6us for this pattern, and HWDGE rings
opened on the scalar queue tax every ACT instruction by ~0.4us.

Measured rel err vs the fp64 reference ~4e-3 against a 2e-2 gate.

Uses bacc.Bacc (not raw bass.Bass): TRN2 instructions carry at most one
semaphore wait, and Bacc.compile()'s generate_event_semaphores pass
legalizes the multi-wait instructions Tile emits.
"""

import numpy as np

N_CORES = 8
B, D, R = 16384, 64, 2048
B_SHARD = B // N_CORES  # 2048
K = D + 4  # 64 data rows + r_sq hi/lo + x_sq hi/lo rows
KP = 128  # K padded to full partition count for 16-engine DMA spread
BT = 128  # batch rows per tile (PSUM partition dim)
RC = 512  # refs cols per matmul (one fp32 PSUM bank)
ACT_COLS = 2048  # Exp activation span: 4 PSUM banks per instruction


def _build_nc():
    from contextlib import ExitStack

    import concourse.tile as tile
    from concourse import bacc, mybir

    f16 = mybir.dt.float16
    bf16 = mybir.dt.bfloat16
    f32 = mybir.dt.float32

    nc = bacc.Bacc(None)
    # x-shard transpose and refs transpose concatenated along the free dim:
    # cols [0, B_SHARD) = (2x).T block, cols [B_SHARD, B_SHARD+R) = refs.T.
    inT_aug = nc.declare_dram_parameter(
        "inT_aug", [KP, B_SHARD + R], f16, isOutput=False
    )
    out = nc.declare_dram_parameter("out", [B_SHARD, R], bf16, isOutput=True)

    n_bt = B_SHARD // BT
    n_rc = R // RC

    with tile.TileContext(nc) as tc, ExitStack() as ctx:
        consts = ctx.enter_context(tc.tile_pool(name="consts", bufs=1))
        outs = ctx.enter_context(tc.tile_pool(name="outs", bufs=4))
        psums = ctx.enter_context(tc.tile_pool(name="psums", bufs=2, space="PSUM"))

        in_sb = consts.tile([KP, B_SHARD + R], f16)
        # Input pieces ordered by first use; they serialize FIFO on the one
        # sync HWDGE ring (~210GB/s), so the bt0 operands (x block 0 + the
        # four refs chunks) ship before the x remainder, and subtile deps
        # release each matmul as its piece lands.
        nc.sync.dma_start(out=in_sb[:, :BT], in_=inT_aug[:, :BT])
        for rc in range(R // RC):
            lo, hi = B_SHARD + rc * RC, B_SHARD + (rc + 1) * RC
            nc.sync.dma_start(out=in_sb[:, lo:hi], in_=inT_aug[:, lo:hi])
        x_rest = B_SHARD - BT  # 1920 cols, in 3 pieces of 640
        for j in range(3):
            lo = BT + j * (x_rest // 3)
            hi = BT + (j + 1) * (x_rest // 3)
            nc.sync.dma_start(out=in_sb[:, lo:hi], in_=inT_aug[:, lo:hi])

        for bt in range(n_bt):
            ps = psums.tile([BT, ACT_COLS], f32)
            out_sb = outs.tile([BT, R], bf16)
            for rc in range(n_rc):
                nc.tensor.matmul(
                    ps[:, rc * RC : (rc + 1) * RC],
                    lhsT=in_sb[:K, bt * BT : (bt + 1) * BT],
                    rhs=in_sb[:K, B_SHARD + rc * RC : B_SHARD + (rc + 1) * RC],
                    start=True,
                    stop=True,
                )
            nc.scalar.activation(
                out_sb,
                ps,
                mybir.ActivationFunctionType.Exp,
                bias=0.0,
                scale=1.0,
            )
            nc.sync.dma_start(out=out[bt * BT : (bt + 1) * BT, :], in_=out_sb)

    nc.compile()
    return nc


def _hi_lo(v):
    """Split fp64 vector into fp16-representable hi + fp16 remainder lo."""
    hi = v.astype(np.float16)
    lo = (v - hi.astype(np.float64)).astype(np.float16)
    return hi, lo


def make_in_maps(x, refs):
    """Host-side prep: shard/transpose x, pack norm terms as extra K rows.

    The x data rows carry 2x so the single K=68 contraction yields
    2*x.r - x_sq - r_sq = -||x - r||^2 directly.
    """
    x = np.ascontiguousarray(x, dtype=np.float32)
    refs = np.ascontiguousarray(refs, dtype=np.float32)

    r_hi, r_lo = _hi_lo((refs.astype(np.float64) ** 2).sum(axis=1))
    x_sq = (x.astype(np.float64) ** 2).sum(axis=1)  # [B]
    xT16 = np.ascontiguousarray((2.0 * x.T).astype(np.float16))  # [D, B]
    rT16 = np.ascontiguousarray(refs.T.astype(np.float16))  # [D, R]

    in_maps = []
    for c in range(N_CORES):
        sl = slice(c * B_SHARD, (c + 1) * B_SHARD)
        x_hi, x_lo = _hi_lo(x_sq[sl])
        inT_aug = np.zeros((KP, B_SHARD + R), np.float16)
        inT_aug[:D, :B_SHARD] = xT16[:, sl]
        inT_aug[D, :B_SHARD] = 1.0
        inT_aug[D + 1, :B_SHARD] = 1.0
        inT_aug[D + 2, :B_SHARD] = -x_hi
        inT_aug[D + 3, :B_SHARD] = -x_lo
        inT_aug[:D, B_SHARD:] = rT16
        inT_aug[D, B_SHARD:] = -r_hi
        inT_aug[D + 1, B_SHARD:] = -r_lo
        inT_aug[D + 2, B_SHARD:] = 1.0
        inT_aug[D + 3, B_SHARD:] = 1.0
        in_maps.append({"inT_aug": inT_aug})
    return in_maps


_NC_CACHE = None


def get_nc():
    global _NC_CACHE
    if _NC_CACHE is None:
        _NC_CACHE = _build_nc()
    return _NC_CACHE


def kernel(x, refs):
    from concourse.bass_utils import run_bass_kernel_spmd

    in_maps = make_in_maps(x, refs)
    res = run_bass_kernel_spmd(
        get_nc(), in_maps, core_ids=list(range(N_CORES))
    ).results
    return np.concatenate(
        [res[c]["out"].astype(np.float32) for c in range(N_CORES)], axis=0
    )


# revision 7
# speedup vs baseline: 1.2548x; 1.0085x over previous
"""RBF kernel feature map: out[b, r] = exp(-||x[b] - refs[r]||^2).

Computed via the GEMM expansion on 8 NeuronCores, data-parallel over the
batch dim of x (2048 rows per core), refs replicated.

Per-core device kernel, one K=66 matmul per [128, 512] PSUM bank:
    psum[b, r] = 2*sum_d x[b,d]*refs[r,d] - r_sq[r]
    out[b, r]  = exp(psum[b, r] - x_sq[b])     (x_sq rides the per-
                                                partition ACT bias AP)

The 2x is folded into the packed x rows; r_sq is split hi/lo across two
extra fp16 K rows; x_sq is exact f32 via the activation bias.  All
matmul operands are fp16 (full-rate PE at the sustained 1.2GHz clock);
the Exp activation covers a full [128, 2048] 4-bank PSUM span per
instruction (ACT cost law measured 260ns + 0.833ns/col) and writes
bf16, halving the dominant output HBM traffic (the host upcasts).

Input DMA: K rows are padded to 128 DRAM rows because DMA engine spread
is partition-driven — [68, n] lands on ~4 of 16 SDMA engines, [128, n]
on all 16.  Pieces go on the single sync HWDGE ring (~280GB/s, FIFO) in
first-use order: x block 0 + refs chunks gate the bt0 matmuls, the x
remainder only gates bt>=1.  (SWDGE descriptor generation measured ~6us
for this pattern; HWDGE rings opened on the scalar queue tax every ACT
instruction ~0.4us — so everything rides the sync ring.)

The last batch tile is split into two half-span activations + DMAs to
shorten the drain tail.  Measured rel err vs fp64 reference ~3.6e-3
against a 2e-2 gate.

Uses bacc.Bacc (not raw bass.Bass): TRN2 instructions carry at most one
semaphore wait, and Bacc.compile()'s generate_event_semaphores pass
legalizes the multi-wait instructions Tile emits.
"""

import numpy as np

N_CORES = 8
B, D, R = 16384, 64, 2048
B_SHARD = B // N_CORES  # 2048
K = D + 2  # 64 data rows + r_sq hi/lo rows (x_sq rides the ACT bias)
KP = 128  # K padded to full partition count for 16-engine DMA spread
BT = 128  # batch rows per tile (PSUM partition dim)
RC = 512  # refs cols per matmul (one fp32 PSUM bank)
ACT_COLS = 2048  # Exp activation span: 4 PSUM banks per instruction
NC_IN = B_SHARD + R  # 4096 input cols: [x blk0 | refs | x blks 1..15]


def _build_nc():
    from contextlib import ExitStack

    import concourse.tile as tile
    from concourse import bacc, mybir

    f16 = mybir.dt.float16
    bf16 = mybir.dt.bfloat16
    f32 = mybir.dt.float32

    nc = bacc.Bacc(None)
    # Free-dim layout: cols [0,128) = (2x).T block 0, [128, 128+R) = refs.T,
    # [128+R, 4096) = (2x).T blocks 1..15.  lhsT for block j>=1 is at
    # col 2048 + 128*j.
    inT_aug = nc.declare_dram_parameter("inT_aug", [KP, NC_IN], f16, isOutput=False)
    xsq_neg = nc.declare_dram_parameter("xsq_neg", [BT, B_SHARD // BT], f32, isOutput=False)
    out = nc.declare_dram_parameter("out", [B_SHARD, R], bf16, isOutput=True)

    n_bt = B_SHARD // BT
    n_rc = R // RC

    with tile.TileContext(nc) as tc, ExitStack() as ctx:
        consts = ctx.enter_context(tc.tile_pool(name="consts", bufs=1))
        outs = ctx.enter_context(tc.tile_pool(name="outs", bufs=4))
        psums = ctx.enter_context(tc.tile_pool(name="psums", bufs=2, space="PSUM"))

        xsq_sb = consts.tile([BT, B_SHARD // BT], f32)
        in_sb = consts.tile([KP, NC_IN], f16)
        # Pieces serialize FIFO on the one sync HWDGE ring; ship in first-use
        # order so subtile deps release each matmul as its piece lands.
        nc.sync.dma_start(out=xsq_sb, in_=xsq_neg[:, :])
        for lo, hi in ((0, 640), (640, 1664), (1664, 2176),
                       (2176, 2816), (2816, 3456), (3456, 4096)):
            nc.sync.dma_start(out=in_sb[:, lo:hi], in_=inT_aug[:, lo:hi])

        def lhsT(bt):
            base = 0 if bt == 0 else B_SHARD + bt * BT
            return in_sb[:K, base : base + BT]

        for bt in range(n_bt):
            ps = psums.tile([BT, ACT_COLS], f32)
            out_sb = outs.tile([BT, R], bf16)
            for rc in range(n_rc):
                nc.tensor.matmul(
                    ps[:, rc * RC : (rc + 1) * RC],
                    lhsT=lhsT(bt),
                    rhs=in_sb[:K, BT + rc * RC : BT + (rc + 1) * RC],
                    start=True,
                    stop=True,
                )
            bias = xsq_sb[:, bt : bt + 1]
            if bt < n_bt - 1:
                nc.scalar.activation(
                    out_sb, ps, mybir.ActivationFunctionType.Exp,
                    bias=bias, scale=1.0,
                )
                nc.sync.dma_start(out=out[bt * BT : (bt + 1) * BT, :], in_=out_sb)
            else:
                # tail: half-span ACTs let the final DMAs start ~1us earlier
                h = ACT_COLS // 2
                for j in range(2):
                    nc.scalar.activation(
                        out_sb[:, j * h : (j + 1) * h],
                        ps[:, j * h : (j + 1) * h],
                        mybir.ActivationFunctionType.Exp,
                        bias=bias, scale=1.0,
                    )
                    nc.sync.dma_start(
                        out=out[bt * BT : (bt + 1) * BT, j * h : (j + 1) * h],
                        in_=out_sb[:, j * h : (j + 1) * h],
                    )

    nc.compile()
    return nc


def _hi_lo(v):
    """Split fp64 vector into fp16-representable hi + fp16 remainder lo."""
    hi = v.astype(np.float16)
    lo = (v - hi.astype(np.float64)).astype(np.float16)
    return hi, lo


def make_in_maps(x, refs):
    """Host-side prep: shard/transpose x, pack refs norms as extra K rows.

    The x data rows carry 2x so the K=66 contraction plus the -x_sq ACT
    bias yields 2*x.r - r_sq - x_sq = -||x - r||^2.
    """
    x = np.ascontiguousarray(x, dtype=np.float32)
    refs = np.ascontiguousarray(refs, dtype=np.float32)

    r_hi, r_lo = _hi_lo((refs.astype(np.float64) ** 2).sum(axis=1))
    x_sq = (x.astype(np.float64) ** 2).sum(axis=1)  # [B]
    xT16 = np.ascontiguousarray((2.0 * x.T).astype(np.float16))  # [D, B]
    rT16 = np.ascontiguousarray(refs.T.astype(np.float16))  # [D, R]

    n_bt = B_SHARD // BT
    in_maps = []
    for c in range(N_CORES):
        sl = slice(c * B_SHARD, (c + 1) * B_SHARD)
        inT_aug = np.zeros((KP, NC_IN), np.float16)
        xc = xT16[:, sl]
        inT_aug[:D, :BT] = xc[:, :BT]
        inT_aug[D, :BT] = 1.0
        inT_aug[D + 1, :BT] = 1.0
        inT_aug[:D, BT : BT + R] = rT16
        inT_aug[D, BT : BT + R] = -r_hi
        inT_aug[D + 1, BT : BT + R] = -r_lo
        inT_aug[:D, BT + R :] = xc[:, BT:]
        inT_aug[D, BT + R :] = 1.0
        inT_aug[D + 1, BT + R :] = 1.0
        xsq_neg = np.ascontiguousarray(
            -x_sq[sl].astype(np.float32).reshape(n_bt, BT).T
        )
        in_maps.append({"inT_aug": inT_aug, "xsq_neg": xsq_neg})
    return in_maps


_NC_CACHE = None


def get_nc():
    global _NC_CACHE
    if _NC_CACHE is None:
        _NC_CACHE = _build_nc()
    return _NC_CACHE


def kernel(x, refs):
    from concourse.bass_utils import run_bass_kernel_spmd

    in_maps = make_in_maps(x, refs)
    res = run_bass_kernel_spmd(
        get_nc(), in_maps, core_ids=list(range(N_CORES))
    ).results
    return np.concatenate(
        [res[c]["out"].astype(np.float32) for c in range(N_CORES)], axis=0
    )
